# revision 1
# baseline (speedup 1.0000x reference)
"""Trainium2 Bass kernel for nn_CausalLaCTSwiGLUWithSlidingWindowAttn.

Reference semantics (B=1, L=8192, DIM=2048, H=16, HEAD_DIM=D_H=128, CHUNK=2048):
  qkv = silu(x @ W_qkv); the (b, l, h, d) -> (b*h, l, d) reshape in the
  reference INTERLEAVES tokens and heads: "group" g's row t corresponds to
  token 512*g + t//16, head t%16.  lr/scale/momentum/w0/w1/w2/output use g as a
  true head index.  Momentum provably cancels (Newton-Schulz normalizes scale),
  so W_m/b_m are unused.

Sharding: 2 groups per core (g = 2c, 2c+1).  Each core computes qkv for its
1024 tokens x all 6144 columns, the thin lr/scale projection for its tokens x
all 64 consumer-ordered columns (AllToAll distributes them), runs the LaCT
chunk recurrence for its 2 groups, and produces a partial (8192, 2048) output
(its groups' rows through W_o).  Host sums the 8 partials.
"""

import math
import os
import numpy as np
from contextlib import ExitStack

import concourse.bass as bass
import concourse.mybir as mybir
import concourse.tile as tile
from concourse import bacc
from concourse.bass_utils import run_bass_kernel_spmd

F32 = mybir.dt.float32
F32R = mybir.dt.float32r
BF16 = mybir.dt.bfloat16
AF = mybir.ActivationFunctionType
OP = mybir.AluOpType

L = 8192
DIM = 2048
H = 16
HD = 128
CHUNK = 2048
NCORES = 8
NT = 8          # token tiles per core (128 tokens each)
BASE_LR = 0.01
BASE_LR_INV = float(BASE_LR + math.log(-math.expm1(-np.float32(BASE_LR))))
NS_COEFS = [(4.0848, -6.8946, 2.927), (3.9505, -6.3029, 2.6377),
            (3.7418, -5.5913, 2.3037), (2.8769, -3.1427, 1.2046),
            (2.8366, -3.0525, 1.2012)]

_CACHED = {}

# 20-node Gauss-Legendre on [0,1]: softplus(z) = ln2 + sum_i w_i * z * sigmoid(u_i z)
#                                              = ln2 + sum_i (w_i/u_i) * silu(u_i z)
import numpy.polynomial.legendre as _leg
_GL_N, _GL_W = _leg.leggauss(20)
GL_U = (0.5 * (_GL_N + 1)).tolist()
GL_WU = (0.5 * _GL_W / (0.5 * (_GL_N + 1))).tolist()
LN2 = float(np.log(2.0))


def build_kernel():
    nc = bacc.Bacc("TRN2", target_bir_lowering=False, debug=False, num_devices=NCORES)

    # ---------------- DRAM I/O ----------------
    xs_d = nc.dram_tensor("xs", [NT * 128, DIM], F32, kind="ExternalInput")
    wq_d = nc.dram_tensor("wq", [DIM, 3 * DIM], F32R, kind="ExternalInput")
    wsm_d = nc.dram_tensor("wsm", [DIM, 64], F32, kind="ExternalInput")
    bs64_d = nc.dram_tensor("bs64", [1, 64], F32, kind="ExternalInput")
    wo_d = nc.dram_tensor("wo", [2 * HD, DIM], F32, kind="ExternalInput")
    w0_d = nc.dram_tensor("w0i", [2 * HD, HD], F32, kind="ExternalInput")
    w1_d = nc.dram_tensor("w1i", [2 * HD, HD], F32, kind="ExternalInput")
    w2_d = nc.dram_tensor("w2i", [2 * HD, HD], F32, kind="ExternalInput")
    lng_d = nc.dram_tensor("lng", [1, HD], F32, kind="ExternalInput")
    lnb_d = nc.dram_tensor("lnb", [1, HD], F32, kind="ExternalInput")
    eye_d = nc.dram_tensor("eye", [128, 128], F32, kind="ExternalInput")
    out_d = nc.dram_tensor("out", [L, DIM], F32, kind="ExternalOutput")

    # internal DRAM
    dbg = os.environ.get("KDEBUG") == "1"
    stage_d = nc.dram_tensor("qkv_stage", [NT, 128, 3 * DIM], F32)
    cc_in = nc.dram_tensor("cc_in", [64, NT * 128], F32)
    cc_out = nc.dram_tensor("cc_out", [64, NT * 128], F32)
    if dbg:
        dbg_ccin = nc.dram_tensor("dbg_ccin", [64, NT * 128], F32, kind="ExternalOutput")
        dbg_ccout = nc.dram_tensor("dbg_ccout", [64, NT * 128], F32, kind="ExternalOutput")
        dbg_lrv = nc.dram_tensor("dbg_lrv", [4, 128, 8, 16], F32, kind="ExternalOutput")
        dbg_qkvn = nc.dram_tensor("dbg_qkvn", [2, 128, H, 3 * HD], F32, kind="ExternalOutput")
        dbg_outT = nc.dram_tensor("dbg_outT", [2, 128, H, HD], F32, kind="ExternalOutput")
        dbg_w = nc.dram_tensor("dbg_w", [2, 128, 3, HD], F32, kind="ExternalOutput")
        dbg_bwd = nc.dram_tensor("dbg_bwd", [8, 128, HD], F32, kind="ExternalOutput")
        dbg_dw = nc.dram_tensor("dbg_dw", [6, 128, HD], F32, kind="ExternalOutput")

    cpy_ctr = [0]

    with tile.TileContext(nc) as tc, ExitStack() as top:
        consts = top.enter_context(tc.tile_pool(name="consts", bufs=1))
        smalls = top.enter_context(tc.tile_pool(name="smalls", bufs=6))
        scr128 = top.enter_context(tc.tile_pool(name="scr128", bufs=3))

        rsq_ctr = [0]

        def emit_rsqrt(pool, out, s):
            # out = 1/sqrt(s) elementwise; quake init + 3 Newton iterations.
            P, Fn = s.shape[0], int(np.prod(s.shape[1:]))
            I32 = mybir.dt.int32
            n = rsq_ctr[0]
            rsq_ctr[0] += 1
            y = pool.tile(list(s.shape), F32, tag="rsq_y", name=f"rsqy{n}")
            nc.vector.tensor_scalar(y.bitcast(I32), s.bitcast(I32), 1, None,
                                    op0=OP.logical_shift_right)
            nc.vector.tensor_scalar(y.bitcast(I32), y.bitcast(I32), -1, 0x5F3759DF,
                                    op0=OP.mult, op1=OP.add)
            h = pool.tile(list(s.shape), F32, tag="rsq_h", name=f"rsqh{n}")
            nc.vector.tensor_scalar_mul(h, s, -0.5)
            for it in range(3):
                t = pool.tile(list(s.shape), F32, tag="rsq_t", name=f"rsqt{n}_{it}")
                nc.vector.tensor_tensor(out=t, in0=y, in1=y, op=OP.mult)
                nc.vector.tensor_tensor(out=t, in0=t, in1=h, op=OP.mult)
                nc.vector.tensor_scalar_add(t, t, 1.5)
                yn = pool.tile(list(s.shape), F32, tag="rsq_y", name=f"rsqy{n}_{it}")
                nc.vector.tensor_tensor(out=yn, in0=y, in1=t, op=OP.mult)
                y = yn
            nc.vector.tensor_copy(out, y)

        def emit_recip_sqrt_eps(pool, out, s, eps):
            # out = 1/(sqrt(s) + eps)
            n = rsq_ctr[0]
            r = pool.tile(list(s.shape), F32, tag="rsq_r", name=f"rsqr{n}")
            emit_rsqrt(pool, r, s)
            sq = pool.tile(list(s.shape), F32, tag="rsq_q", name=f"rsqq{n}")
            nc.vector.tensor_tensor(out=sq, in0=s, in1=r, op=OP.mult)
            nc.vector.tensor_scalar_add(sq, sq, eps)
            nc.vector.reciprocal(out, sq)

        def altcopy(dst, src):
            # alternate psum->sbuf evacuation between ACT and DVE
            if cpy_ctr[0] % 2 == 0:
                nc.scalar.copy(dst, src)
            else:
                nc.vector.tensor_copy(dst, src)
            cpy_ctr[0] += 1

        # ---------------- constants ----------------
        eye_f = consts.tile([128, 128], F32)
        nc.sync.dma_start(out=eye_f, in_=eye_d[:, :])
        eye_r = consts.tile([128, 128], F32R)
        nc.vector.tensor_copy(eye_r, eye_f)
        eye_b = consts.tile([128, 128], BF16)
        nc.vector.tensor_copy(eye_b, eye_f)
        lng_b = consts.tile([128, HD], F32)
        nc.gpsimd.dma_start(out=lng_b, in_=bass.AP(lng_d, 0, [[0, 128], [1, HD]]))
        lnb_b = consts.tile([128, HD], F32)
        nc.gpsimd.dma_start(out=lnb_b, in_=bass.AP(lnb_d, 0, [[0, 128], [1, HD]]))
        ones_col = consts.tile([128, 1], F32)
        nc.vector.memset(ones_col, 1.0)
        ones_row = consts.tile([1, 128], F32)
        nc.vector.memset(ones_row, 1.0)
        ones_row_r = consts.tile([1, 128], F32R)
        nc.vector.tensor_copy(ones_row_r, ones_row)
        bs64_f = consts.tile([1, 64], F32)
        nc.sync.dma_start(out=bs64_f, in_=bs64_d[:, :])
        bs64_r = consts.tile([1, 64], F32R)
        nc.vector.tensor_copy(bs64_r, bs64_f)
        blr_bias = consts.tile([128, 1], F32)
        nc.vector.memset(blr_bias, BASE_LR_INV)
        eps6 = consts.tile([128, 1], F32)
        nc.vector.memset(eps6, 1e-6)

        # wo (256, 2048) -> (128, 2, 2048) f32r
        wo_sb = consts.tile([128, 2, DIM], F32R)
        nc.gpsimd.dma_start(out=wo_sb, in_=wo_d.ap().rearrange("(j p) n -> p j n", p=128))

        # ---------------- fast-weight state ----------------
        # w_sb[j]: (128, 3, 128) [w0, w1, w2] rows-major f32r
        # wT_sb[j]: (128, 3, 128) [w0T, w2T, w1T] f32r
        # n0_sb[j]: (128, 3) initial row norms f32
        w_sb, wT_sb, n0_sb = [], [], []
        for j in range(2):
            w_sb.append(consts.tile([128, 3, HD], F32R, tag=f"wsb{j}", name=f"wsb{j}"))
            wT_sb.append(consts.tile([128, 3, HD], F32R, tag=f"wTsb{j}", name=f"wTsb{j}"))
            n0_sb.append(consts.tile([128, 3], F32, tag=f"n0sb{j}", name=f"n0sb{j}"))
        with tc.tile_pool(name="winit", bufs=2) as winit, \
             tc.tile_pool(name="winit_ps", bufs=2, space="PSUM") as winit_ps:
            for wi, wd in enumerate((w0_d, w1_d, w2_d)):
                wt = winit.tile([128, 2, HD], F32, tag="wload")
                nc.sync.dma_start(out=wt, in_=wd.ap().rearrange("(j p) n -> p j n", p=128))
                for j in range(2):
                    # initial row norms
                    scr = scr128.tile([128, HD], F32, tag="wscr")
                    ss = smalls.tile([128, 1], F32, tag="wss")
                    nc.scalar.activation(scr, wt[:, j, :], AF.Square, accum_out=ss)
                    r0 = smalls.tile([128, 1], F32, tag="wr0", name=f"wr0_{wi}_{j}")
                    emit_rsqrt(smalls, r0, ss)
                    nc.vector.tensor_tensor(out=n0_sb[j][:, wi : wi + 1], in0=ss,
                                            in1=r0, op=OP.mult)
                    # rounded state
                    nc.vector.tensor_copy(w_sb[j][:, wi, :], wt[:, j, :])
                    # transpose
                    ps = winit_ps.tile([128, HD], F32R, tag="wtp")
                    nc.tensor.transpose(ps, w_sb[j][:, wi, :], eye_r)
                    slot = {0: 0, 2: 1, 1: 2}[wi]
                    altcopy(wT_sb[j][:, slot, :], ps)

        # ================= PHASE A =================
        _sk = os.environ.get
        with ExitStack() as phA:
            xpool = phA.enter_context(tc.tile_pool(name="xpool", bufs=3))
            xT_pool = phA.enter_context(tc.tile_pool(name="xTpool", bufs=1))
            xT = xT_pool.tile([128, NT, 16, 128], F32R)
            wsm_sb = xT_pool.tile([128, 16, 64], F32R, name="wsm_sb")
            nc.gpsimd.dma_start(out=wsm_sb, in_=wsm_d.ap().rearrange("(k p) n -> p k n", p=128))

            with tc.tile_pool(name="tpA_ps", bufs=2, space="PSUM") as tpA_ps, \
                 tc.tile_pool(name="thin_ps", bufs=2, space="PSUM") as thin_ps, \
                 tc.tile_pool(name="thin_sb", bufs=2) as thin_sb:
                for t in range(NT) if _sk("KSKIP_PHA") != "1" else []:
                    xt = xpool.tile([128, DIM], F32, tag="xt")
                    nc.sync.dma_start(out=xt, in_=xs_d[t * 128 : (t + 1) * 128, :])
                    for k in range(16):
                        ps = tpA_ps.tile([128, 128], F32, tag="tp")
                        nc.tensor.transpose(ps, xt[:, k * 128 : (k + 1) * 128], eye_f)
                        altcopy(xT[:, t, k, :], ps)
                # thin projection (feature-major): psum (64, 128) per tile
                for t in range(NT) if _sk("KSKIP_PHA") != "1" else []:
                    pst = thin_ps.tile([64, 128], F32, tag="thin")
                    for k in range(16):
                        nc.tensor.matmul(pst, wsm_sb[:, k, :], xT[:, t, k, :],
                                         start=(k == 0), stop=False)
                    # + b_s (row-constant bias on scale rows; zero elsewhere)
                    nc.tensor.matmul(pst, bs64_r, ones_row_r, start=False, stop=True)
                    tsb = thin_sb.tile([64, 128], F32, tag="tsb")
                    altcopy(tsb, pst)
                    nc.sync.dma_start(out=cc_in[:, t * 128 : (t + 1) * 128], in_=tsb)
                if os.environ.get("KNOCC") == "1":
                    # timing-model build: collective replaced by a local DRAM copy
                    nc.sync.dma_start(out=cc_out[:, :], in_=cc_in[:, :])
                else:
                    nc.gpsimd.collective_compute(
                        "AllToAll", OP.bypass,
                        replica_groups=[list(range(NCORES))],
                        ins=[cc_in.ap()], outs=[cc_out.ap()],
                    )
                if dbg:
                    nc.sync.dma_start(out=dbg_ccin[:, :], in_=cc_in[:, :])
                    nc.sync.dma_start(out=dbg_ccout[:, :], in_=cc_out[:, :])

            # qkv projection: loop over head column blocks
            with tc.tile_pool(name="wqpool", bufs=2) as wqpool, \
                 tc.tile_pool(name="qkv_ps", bufs=4, space="PSUM") as qkv_ps, \
                 tc.tile_pool(name="stpool", bufs=3) as stpool:
                wq_r = wq_d.ap().rearrange("(k p) n -> p k n", p=128)
                for m in range(H) if _sk("KSKIP_PHA") != "1" else []:
                    wq_m = wqpool.tile([128, 16, 3 * HD], F32R, tag="wqm")
                    nc.sync.dma_start(out=wq_m, in_=wq_r[:, :, m * 384 : (m + 1) * 384])
                    for t in range(NT):
                        psq = qkv_ps.tile([128, 3 * HD], F32, tag="psq")
                        for k in range(16):
                            nc.tensor.matmul(psq, xT[:, t, k, :], wq_m[:, k, :],
                                             start=(k == 0), stop=(k == 15))
                        st = stpool.tile([128, 3 * HD], F32, tag="st")
                        nc.scalar.activation(st, psq, AF.Silu)
                        nc.sync.dma_start(
                            out=stage_d[t, :, m * 384 : (m + 1) * 384], in_=st)

        # ================= LaCT rounds =================
        with tc.tile_pool(name="lrpool", bufs=2) as lrpool, \
             tc.tile_pool(name="qkvt_pool", bufs=2) as qkvt_pool, \
             tc.tile_pool(name="fm2pool", bufs=1) as fm2pool, \
             tc.tile_pool(name="fmpool", bufs=1) as fmpool, \
             tc.tile_pool(name="owpool", bufs=2) as owpool, \
             tc.tile_pool(name="scr512", bufs=3) as scr512, \
             tc.tile_pool(name="rnd128", bufs=5) as rnd128, \
             tc.tile_pool(name="ps_a", bufs=4, space="PSUM") as ps_a, \
             tc.tile_pool(name="ps_wo", bufs=1, space="PSUM") as ps_wo, \
             tc.tile_pool(name="ps_dw", bufs=1, space="PSUM") as ps_dw:

            lrvs = []
            for ci in range(4):
                lrt = lrpool.tile([128, 8, 16], F32, tag="lrt")
                for b in range(2):
                    src = bass.AP(cc_out, 16384 * ci + 8192 * b,
                                  [[16, 64], [1024, 8], [1, 16]])
                    nc.sync.dma_start(out=lrt[64 * b : 64 * (b + 1), :, :], in_=src)
                lrv = lrpool.tile([128, 8, 16], F32, tag=f"lrv{ci}", name=f"lrv{ci}")
                # z = raw + base_lr_inv; softplus(z) = ln2 + sum (w/u) silu(u z)
                zt = lrpool.tile([128, 6, 16], F32, tag="zt")
                nc.vector.tensor_scalar_add(zt, lrt[:, 0:6, :], BASE_LR_INV)
                acc = lrpool.tile([128, 6, 16], F32, tag="spacc")
                nc.vector.memset(acc, LN2)
                for u_i, wu_i in zip(GL_U, GL_WU):
                    tmp = lrpool.tile([128, 6, 16], F32, tag="sptmp")
                    nc.scalar.activation(tmp, zt, AF.Silu, scale=float(u_i))
                    acc2 = lrpool.tile([128, 6, 16], F32, tag="spacc")
                    nc.vector.scalar_tensor_tensor(out=acc2, in0=tmp,
                                                   scalar=float(wu_i), in1=acc,
                                                   op0=OP.mult, op1=OP.add)
                    acc = acc2
                nc.vector.tensor_copy(lrv[:, 0:6, :], acc)
                nc.scalar.activation(lrv[:, 6:8, :], lrt[:, 6:8, :], AF.Silu)
                if dbg:
                    nc.sync.dma_start(out=dbg_lrv[ci], in_=lrv)
                lrvs.append(lrv)

            for ci in range(4) if os.environ.get("KSKIP_ROUNDS") != "1" else []:
                lrv = lrvs[ci]

                outT_tiles = []
                for j in range(2):
                    tidx = j * 4 + ci
                    qkvt = qkvt_pool.tile([128, H, 3 * HD], F32R, tag="qkvt")
                    nc.gpsimd.dma_start(out=qkvt, in_=stage_d[tidx])
                    # l2-normalize q,k rows (batched rsqrt over 32 norms)
                    ssb = scr128.tile([128, 32], F32, tag="ssb")
                    for m in range(H):
                        for ci2, c0 in ((0, 0), (1, 128)):
                            nscr = scr128.tile([128, 128], F32, tag="nscr")
                            nc.scalar.activation(nscr, qkvt[:, m, c0 : c0 + 128],
                                                 AF.Square,
                                                 accum_out=ssb[:, 2 * m + ci2 : 2 * m + ci2 + 1])
                    nfac = scr128.tile([128, 32], F32, tag="nfac")
                    emit_recip_sqrt_eps(scr128, nfac, ssb, 1e-5)
                    for m in range(H):
                        nc.vector.tensor_scalar_mul(qkvt[:, m, 0:128], qkvt[:, m, 0:128],
                                                    nfac[:, 2 * m : 2 * m + 1])
                        nc.vector.tensor_scalar_mul(qkvt[:, m, 128:256],
                                                    qkvt[:, m, 128:256],
                                                    nfac[:, 2 * m + 1 : 2 * m + 2])
                    if dbg and ci == 0:
                        nc.gpsimd.dma_start(out=dbg_qkvn[j], in_=qkvt)
                    qT = fm2pool.tile([128, H, HD], F32R, tag="qT")
                    kT = fm2pool.tile([128, H, HD], F32R, tag="kT")
                    vT = fm2pool.tile([128, H, HD], F32R, tag="vT")
                    for m in range(H):
                        for c0, dst in ((0, qT), (128, kT), (256, vT)):
                            ps = ps_a.tile([128, 128], F32R, tag="ps")
                            nc.tensor.transpose(ps, qkvt[:, m, c0 : c0 + 128], eye_r)
                            altcopy(dst[:, m, :], ps)

                    # ---- forward ----
                    gh = fm2pool.tile([128, H, HD], F32R, tag="gh")
                    for s in range(4):
                        psg = ps_a.tile([128, 512], F32, tag="ps")
                        psh = ps_a.tile([128, 512], F32, tag="ps")
                        rhs = qT[:, 4 * s : 4 * (s + 1), :]
                        nc.tensor.matmul(psg, wT_sb[j][:, 0, :], rhs, start=True, stop=True)
                        nc.tensor.matmul(psh, wT_sb[j][:, 1, :], rhs, start=True, stop=True)
                        sg4 = scr512.tile([128, 512], F32, tag="sg4")
                        nc.scalar.activation(sg4, psg, AF.Silu)
                        nc.vector.tensor_tensor(
                            out=gh[:, 4 * s : 4 * (s + 1), :], in0=sg4, in1=psh,
                            op=OP.mult)

                    outT = fmpool.tile([128, H, HD], F32R, tag=f"outT{j}")
                    outn = fmpool.tile([128, H, HD], F32, tag="outn")
                    mvb = fmpool.tile([128, H, 2], F32, tag="mvb")
                    for m in range(H):
                        pso = ps_a.tile([128, 128], F32, tag="ps")
                        nc.tensor.matmul(pso, gh[:, m, :], wT_sb[j][:, 2, :],
                                         start=True, stop=True)
                        st6 = smalls.tile([128, 6], F32, tag="st6")
                        nc.vector.bn_stats(st6, pso)
                        nc.vector.bn_aggr(mvb[:, m, :], st6)
                        nc.vector.tensor_scalar(outn[:, m, :], pso, mvb[:, m, 0:1],
                                                None, op0=OP.subtract)
                    vb = scr128.tile([128, H], F32, tag="vb")
                    nc.vector.tensor_scalar_add(vb, mvb[:, :, 1], 1e-6)
                    rstdb = scr128.tile([128, H], F32, tag="rstdb")
                    emit_rsqrt(scr128, rstdb, vb)
                    for m in range(H):
                        tn = scr128.tile([128, 128], F32, tag="tn")
                        nc.vector.scalar_tensor_tensor(
                            out=tn, in0=outn[:, m, :], scalar=rstdb[:, m : m + 1],
                            in1=lng_b, op0=OP.mult, op1=OP.mult)
                        nc.vector.tensor_tensor(out=tn, in0=tn, in1=lnb_b, op=OP.add)
                        tns = rnd128.tile([128, 128], F32R, tag="tns")
                        nc.vector.tensor_scalar_mul(tns, tn, lrv[:, 6 + j, m : m + 1])
                        pst = ps_a.tile([128, 128], F32R, tag="ps")
                        nc.tensor.transpose(pst, tns, eye_r)
                        altcopy(outT[:, m, :], pst)
                    if dbg and ci == 0:
                        nc.gpsimd.dma_start(out=dbg_outT[j], in_=outT)
                    outT_tiles.append(outT)

                    # ---- backward + NS + weight update ----
                    if ci < 3 and os.environ.get("KSKIP_BWD") != "1":
                        dwps = [ps_dw.tile([128, HD], F32, tag=f"dw{i}", name=f"dwp{i}_{ci}_{j}")
                                for i in range(3)]
                        for m in range(H):
                            ps3 = ps_a.tile([128, 384], F32, tag="ps")
                            nc.tensor.matmul(ps3[:, 0:256], kT[:, m, :],
                                             wT_sb[j][:, 0:2, :], start=True, stop=True)
                            nc.tensor.matmul(ps3[:, 256:384], vT[:, m, :],
                                             w_sb[j][:, 1, :], start=True, stop=True)
                            th = scr128.tile([128, 128], F32, tag="th")
                            nc.scalar.activation(th, ps3[:, 0:128], AF.Tanh, scale=0.5)
                            sig = scr128.tile([128, 128], F32, tag="sig")
                            nc.vector.tensor_scalar(sig, th, 0.5, 0.5, op0=OP.mult,
                                                    op1=OP.add)
                            sg = scr128.tile([128, 128], F32, tag="sg")
                            nc.vector.tensor_tensor(out=sg, in0=ps3[:, 0:128], in1=sig,
                                                    op=OP.mult)
                            du = scr128.tile([128, 128], F32, tag="du")
                            nc.vector.tensor_tensor(out=du, in0=ps3[:, 0:128], in1=sg,
                                                    op=OP.subtract)
                            nc.vector.tensor_scalar_add(du, du, 1.0)
                            ds = scr128.tile([128, 128], F32, tag="ds")
                            nc.vector.tensor_tensor(out=ds, in0=sig, in1=du, op=OP.mult)
                            hb_sb = scr128.tile([128, 128], F32, tag="hbsb")
                            altcopy(hb_sb, ps3[:, 128:256])
                            hid_lr = rnd128.tile([128, 128], F32R, tag="hidlr")
                            nc.vector.scalar_tensor_tensor(
                                out=hid_lr, in0=sg, scalar=lrv[:, 3 * j + 1, m : m + 1],
                                in1=hb_sb, op0=OP.mult, op1=OP.mult)
                            tdg = scr128.tile([128, 128], F32, tag="tdg")
                            nc.vector.tensor_tensor(out=tdg, in0=ps3[:, 256:384],
                                                    in1=hb_sb, op=OP.mult)
                            dgb_lr = rnd128.tile([128, 128], F32R, tag="dgblr")
                            nc.vector.scalar_tensor_tensor(
                                out=dgb_lr, in0=tdg, scalar=lrv[:, 3 * j + 0, m : m + 1],
                                in1=ds, op0=OP.mult, op1=OP.mult)
                            dhb_lr = rnd128.tile([128, 128], F32R, tag="dhblr")
                            nc.vector.scalar_tensor_tensor(
                                out=dhb_lr, in0=ps3[:, 256:384],
                                scalar=lrv[:, 3 * j + 2, m : m + 1],
                                in1=sg, op0=OP.mult, op1=OP.mult)
                            if dbg and ci == 0 and j == 0 and m == 0:
                                for di, dt_ in enumerate((sg, ds, hb_sb, tdg)):
                                    nc.sync.dma_start(out=dbg_bwd[di], in_=dt_)
                                for di, dt_ in enumerate((hid_lr, dgb_lr, dhb_lr)):
                                    nc.gpsimd.dma_start(out=dbg_bwd[4 + di], in_=dt_)
                                gbc = scr128.tile([128, 128], F32, tag="gbc")
                                nc.vector.tensor_copy(gbc, ps3[:, 128:256])
                                nc.sync.dma_start(out=dbg_bwd[7], in_=gbc)
                            k_tm = qkvt[:, m, 128:256]
                            v_tm = qkvt[:, m, 256:384]
                            nc.tensor.matmul(dwps[0], dgb_lr, k_tm,
                                             start=(m == 0), stop=(m == 15))
                            nc.tensor.matmul(dwps[1], v_tm, hid_lr,
                                             start=(m == 0), stop=(m == 15))
                            nc.tensor.matmul(dwps[2], dhb_lr, k_tm,
                                             start=(m == 0), stop=(m == 15))

                        if dbg and ci == 0 and j == 0:
                            for wi2 in range(3):
                                dcp = scr128.tile([128, 128], F32, tag="dcp", name=f"dcp{wi2}")
                                nc.vector.tensor_copy(dcp, dwps[wi2])
                                nc.sync.dma_start(out=dbg_dw[wi2], in_=dcp)
                        if os.environ.get("KSKIP_NS") == "1":
                            continue
                        fsb = smalls.tile([128, 3], F32, tag="fsb")
                        for wi in range(3):
                            scr = scr128.tile([128, 128], F32, tag="nsscr")
                            nc.scalar.activation(scr, dwps[wi], AF.Square,
                                                 accum_out=fsb[:, wi : wi + 1])
                        ps13 = ps_a.tile([1, 3], F32, tag="ps")
                        nc.tensor.matmul(ps13, ones_col, fsb, start=True, stop=True)
                        s13 = smalls.tile([1, 3], F32, tag="s13")
                        nc.vector.tensor_copy(s13, ps13)
                        r13 = smalls.tile([1, 3], F32, tag="r13")
                        emit_recip_sqrt_eps(smalls, r13, s13, 1e-7)
                        psb3 = ps_a.tile([128, 3], F32, tag="ps")
                        nc.tensor.matmul(psb3, ones_row, r13, start=True, stop=True)
                        rbs3 = smalls.tile([128, 3], F32, tag="rbs3")
                        altcopy(rbs3, psb3)
                        for wi in range(3):
                            X = rnd128.tile([128, 128], F32R, tag="X")
                            nc.vector.tensor_scalar_mul(X, dwps[wi], rbs3[:, wi : wi + 1])
                            # XT seed via one transpose; thereafter dual-tracked
                            pst = ps_a.tile([128, 128], F32R, tag="ps")
                            nc.tensor.transpose(pst, X, eye_r)
                            XT = rnd128.tile([128, 128], F32R, tag="XT")
                            altcopy(XT, pst)
                            for it, (a_i, b_i, c_i) in enumerate(NS_COEFS):
                                last = it == len(NS_COEFS) - 1
                                psA = ps_a.tile([128, 128], F32, tag="ps")
                                nc.tensor.matmul(psA, XT, XT, start=True, stop=True)
                                A_sb = rnd128.tile([128, 128], F32R, tag="Asb")
                                altcopy(A_sb, psA)
                                psA2 = ps_a.tile([128, 128], F32, tag="ps")
                                nc.tensor.matmul(psA2, A_sb, A_sb, start=True, stop=True)
                                t1 = scr128.tile([128, 128], F32, tag="t1")
                                nc.scalar.mul(t1, psA2, c_i)
                                Bm = rnd128.tile([128, 128], F32R, tag="Bm")
                                nc.vector.scalar_tensor_tensor(
                                    out=Bm, in0=psA, scalar=b_i, in1=t1,
                                    op0=OP.mult, op1=OP.add)
                                psBX = ps_a.tile([128, 128], F32, tag="ps")
                                nc.tensor.matmul(psBX, Bm, X, start=True, stop=True)
                                Xn = rnd128.tile([128, 128], F32R, tag="X")
                                nc.vector.scalar_tensor_tensor(
                                    out=Xn, in0=X, scalar=a_i, in1=psBX,
                                    op0=OP.mult, op1=OP.add)
                                if not last:
                                    # XT' = a*XT + (Bm@X)^T = a*XT + X^T@Bm
                                    psXTB = ps_a.tile([128, 128], F32, tag="ps")
                                    nc.tensor.matmul(psXTB, X, Bm, start=True, stop=True)
                                    XTn = rnd128.tile([128, 128], F32R, tag="XT")
                                    nc.vector.scalar_tensor_tensor(
                                        out=XTn, in0=XT, scalar=a_i, in1=psXTB,
                                        op0=OP.mult, op1=OP.add)
                                    XT = XTn
                                X = Xn
                            # w += X; renormalize rows to initial norms
                            if dbg and ci == 0 and j == 0:
                                nc.gpsimd.dma_start(out=dbg_dw[3 + wi], in_=X)
                            wtmp = scr128.tile([128, 128], F32, tag="wtmp")
                            nc.vector.tensor_tensor(out=wtmp, in0=w_sb[j][:, wi, :],
                                                    in1=X, op=OP.add)
                            scr2 = scr128.tile([128, 128], F32, tag="nsscr")
                            ss2 = smalls.tile([128, 1], F32, tag="ss2")
                            nc.scalar.activation(scr2, wtmp, AF.Square, accum_out=ss2)
                            rc2 = smalls.tile([128, 1], F32, tag="rc2")
                            emit_recip_sqrt_eps(smalls, rc2, ss2, 1e-5)
                            fac = smalls.tile([128, 1], F32, tag="fac")
                            nc.vector.tensor_tensor(out=fac, in0=rc2,
                                                    in1=n0_sb[j][:, wi : wi + 1],
                                                    op=OP.mult)
                            nc.vector.tensor_scalar_mul(w_sb[j][:, wi, :], wtmp, fac)
                            pswt = ps_a.tile([128, 128], F32R, tag="ps")
                            nc.tensor.transpose(pswt, w_sb[j][:, wi, :], eye_r)
                            slot = {0: 0, 2: 1, 1: 2}[wi]
                            altcopy(wT_sb[j][:, slot, :], pswt)

                if dbg and ci == 0:
                    for j in range(2):
                        nc.gpsimd.dma_start(out=dbg_w[j], in_=w_sb[j])
                # ---- W_o partial output ----
                for m in range(H) if os.environ.get("KSKIP_WO") != "1" else []:
                    ow = owpool.tile([128, DIM], F32, tag="ow")
                    for n in range(4):
                        psw = ps_wo.tile([128, 512], F32, tag="wo")
                        for j in range(2):
                            nc.tensor.matmul(psw, outT_tiles[j][:, m, :],
                                             wo_sb[:, j, n * 512 : (n + 1) * 512],
                                             start=(j == 0), stop=(j == 1))
                        altcopy(ow[:, n * 512 : (n + 1) * 512], psw)
                    dst = bass.AP(out_d, (2048 * ci + m) * DIM, [[16 * DIM, 128], [1, DIM]])
                    nc.sync.dma_start(out=dst, in_=ow)

    if os.environ.get("KSKIPCOMPILE") != "1":
        nc.compile()
    return nc


def _shard_inputs(x, W_qkv, W_lr, W_o, W_s, b_s, ln_g, ln_b, w0, w1, w2):
    x = np.ascontiguousarray(np.asarray(x, dtype=np.float32).reshape(L, DIM))
    W_qkv = np.ascontiguousarray(np.asarray(W_qkv, dtype=np.float32))
    W_lr = np.asarray(W_lr, dtype=np.float32)
    W_s = np.asarray(W_s, dtype=np.float32)
    b_s = np.asarray(b_s, dtype=np.float32)
    W_o = np.asarray(W_o, dtype=np.float32)
    ln_g = np.ascontiguousarray(np.asarray(ln_g, dtype=np.float32).reshape(1, HD))
    ln_b = np.ascontiguousarray(np.asarray(ln_b, dtype=np.float32).reshape(1, HD))
    w0 = np.asarray(w0, dtype=np.float32)
    w1 = np.asarray(w1, dtype=np.float32)
    w2 = np.asarray(w2, dtype=np.float32)
    eye = np.eye(128, dtype=np.float32)

    wsm_full = np.concatenate([W_lr, W_s], axis=1)  # (2048, 64)
    cols = []
    bs64 = np.zeros((1, 64), dtype=np.float32)
    for c in range(NCORES):
        g0, g1 = 2 * c, 2 * c + 1
        blk = [g0, 16 + g0, 32 + g0, g1, 16 + g1, 32 + g1, 48 + g0, 48 + g1]
        cols.extend(blk)
        bs64[0, 8 * c + 6] = b_s[g0]
        bs64[0, 8 * c + 7] = b_s[g1]
    wsm = np.ascontiguousarray(wsm_full[:, cols])

    in_maps = []
    for c in range(NCORES):
        g0, g1 = 2 * c, 2 * c + 1
        in_maps.append({
            "xs": np.ascontiguousarray(x[1024 * c : 1024 * (c + 1)]),
            "wq": W_qkv,
            "wsm": wsm,
            "bs64": bs64,
            "wo": np.ascontiguousarray(W_o[np.r_[g0 * HD:(g0 + 1) * HD,
                                               g1 * HD:(g1 + 1) * HD], :]),
            "w0i": np.ascontiguousarray(np.concatenate([w0[g0], w0[g1]], axis=0)),
            "w1i": np.ascontiguousarray(np.concatenate([w1[g0], w1[g1]], axis=0)),
            "w2i": np.ascontiguousarray(np.concatenate([w2[g0], w2[g1]], axis=0)),
            "lng": ln_g,
            "lnb": ln_b,
            "eye": eye,
        })
    return in_maps


def kernel(x, W_qkv, W_lr, W_o, W_m, b_m, W_s, b_s, ln_g, ln_b, w0, w1, w2):
    # W_m / b_m intentionally unused: momentum rescales dw by (1 + m) > 0 before
    # Newton-Schulz, which normalizes away any positive scalar factor.
    if "nc" not in _CACHED:
        _CACHED["nc"] = build_kernel()
    nc = _CACHED["nc"]
    in_maps = _shard_inputs(x, W_qkv, W_lr, W_o, W_s, b_s, ln_g, ln_b, w0, w1, w2)
    res = run_bass_kernel_spmd(nc, in_maps, core_ids=list(range(NCORES)))
    out = np.zeros((L, DIM), dtype=np.float64)
    for c in range(NCORES):
        out += res.results[c]["out"]
    return out.astype(np.float32).reshape(1, L, DIM)



# revision 39
# speedup vs baseline: 1.4800x; 1.4800x over previous
"""Trainium2 Bass kernel for nn_CausalLaCTSwiGLUWithSlidingWindowAttn.

Reference semantics (B=1, L=8192, DIM=2048, H=16, HEAD_DIM=D_H=128, CHUNK=2048):
  qkv = silu(x @ W_qkv); the (b, l, h, d) -> (b*h, l, d) reshape in the
  reference INTERLEAVES tokens and heads: "group" g's row t corresponds to
  token 512*g + t//16, head t%16.  lr/scale/momentum/w0/w1/w2/output use g as a
  true head index.  Momentum provably cancels (Newton-Schulz normalizes scale),
  so W_m/b_m are unused.

Sharding: 2 groups per core (g = 2c, 2c+1).  Each core computes qkv for its
1024 tokens x all 6144 columns, the thin lr/scale projection for its tokens x
all 64 consumer-ordered columns (AllToAll distributes them), runs the LaCT
chunk recurrence for its 2 groups, and produces a partial (8192, 2048) output
(its groups' rows through W_o).  Host sums the 8 partials.
"""

import math
import os
import numpy as np
from contextlib import ExitStack

import concourse.bass as bass
import concourse.mybir as mybir
import concourse.tile as tile
from concourse import bacc
from concourse.bass_utils import run_bass_kernel_spmd

F32 = mybir.dt.float32
F32R = mybir.dt.float32r
BF16 = mybir.dt.float16
AF = mybir.ActivationFunctionType
OP = mybir.AluOpType

L = 8192
DIM = 2048
H = 16
HD = 128
CHUNK = 2048
NCORES = 8
NT = 8          # token tiles per core (128 tokens each)
BASE_LR = 0.01
BASE_LR_INV = float(BASE_LR + math.log(-math.expm1(-np.float32(BASE_LR))))
NS_COEFS = [(4.0848, -6.8946, 2.927), (3.9505, -6.3029, 2.6377),
            (3.7418, -5.5913, 2.3037), (2.8769, -3.1427, 1.2046),
            (2.8366, -3.0525, 1.2012)]

_CACHED = {}

# 20-node Gauss-Legendre on [0,1]: softplus(z) = ln2 + sum_i w_i * z * sigmoid(u_i z)
#                                              = ln2 + sum_i (w_i/u_i) * silu(u_i z)
import numpy.polynomial.legendre as _leg
_GL_N, _GL_W = _leg.leggauss(20)
GL_U = (0.5 * (_GL_N + 1)).tolist()
GL_WU = (0.5 * _GL_W / (0.5 * (_GL_N + 1))).tolist()
LN2 = float(np.log(2.0))


def build_kernel():
    nc = bacc.Bacc("TRN2", target_bir_lowering=False, debug=False, num_devices=NCORES)

    # ---------------- DRAM I/O ----------------
    xs_d = nc.dram_tensor("xs", [NT * 128, DIM], BF16, kind="ExternalInput")
    wq_d = nc.dram_tensor("wq", [DIM, 3 * DIM], BF16, kind="ExternalInput")
    wsm_d = nc.dram_tensor("wsm", [DIM, 64], BF16, kind="ExternalInput")
    bs64_d = nc.dram_tensor("bs64", [1, 64], F32, kind="ExternalInput")
    wo_d = nc.dram_tensor("wo", [2 * HD, DIM], BF16, kind="ExternalInput")
    w0_d = nc.dram_tensor("w0i", [2 * HD, HD], F32, kind="ExternalInput")
    w1_d = nc.dram_tensor("w1i", [2 * HD, HD], F32, kind="ExternalInput")
    w2_d = nc.dram_tensor("w2i", [2 * HD, HD], F32, kind="ExternalInput")
    lng_d = nc.dram_tensor("lng", [1, HD], F32, kind="ExternalInput")
    lnb_d = nc.dram_tensor("lnb", [1, HD], F32, kind="ExternalInput")
    eye_d = nc.dram_tensor("eye", [128, 128], F32, kind="ExternalInput")
    out_d = nc.dram_tensor("out", [L, DIM], BF16, kind="ExternalOutput")

    # internal DRAM
    dbg = os.environ.get("KDEBUG") == "1"
    stage_d = nc.dram_tensor("qkv_stage", [NT, 128, 3 * DIM], BF16)
    cc_in = nc.dram_tensor("cc_in", [64, NT * 128], F32)
    cc_out = nc.dram_tensor("cc_out", [64, NT * 128], F32)
    if dbg:
        dbg_ccin = nc.dram_tensor("dbg_ccin", [64, NT * 128], F32, kind="ExternalOutput")
        dbg_ccout = nc.dram_tensor("dbg_ccout", [64, NT * 128], F32, kind="ExternalOutput")
        dbg_lrv = nc.dram_tensor("dbg_lrv", [4, 128, 8, 16], F32, kind="ExternalOutput")
        dbg_qkvn = nc.dram_tensor("dbg_qkvn", [2, 128, H, 3 * HD], F32, kind="ExternalOutput")
        dbg_outT = nc.dram_tensor("dbg_outT", [2, 128, H, HD], F32, kind="ExternalOutput")
        dbg_w = nc.dram_tensor("dbg_w", [2, 128, 3, HD], F32, kind="ExternalOutput")
        dbg_bwd = nc.dram_tensor("dbg_bwd", [8, 128, HD], F32, kind="ExternalOutput")
        dbg_dw = nc.dram_tensor("dbg_dw", [6, 128, HD], F32, kind="ExternalOutput")

    cpy_ctr = [0]

    with tile.TileContext(nc) as tc, ExitStack() as top:
        consts = top.enter_context(tc.tile_pool(name="consts", bufs=1))
        smalls = top.enter_context(tc.tile_pool(name="smalls", bufs=6))
        scr128 = top.enter_context(tc.tile_pool(name="scr128", bufs=2))

        _rsq_n = [0]

        def emit_rsqrt(pool, out, s):
            # out = 1/sqrt(s): quake seed + 3 Newton iterations on DVE
            I32 = mybir.dt.int32
            n = _rsq_n[0]
            _rsq_n[0] += 1
            shp = list(s.shape)
            y = pool.tile(shp, F32, tag="rsq_y", name=f"irsqy{n}")
            nc.vector.tensor_scalar(y.bitcast(I32), s.bitcast(I32), 1, None,
                                    op0=OP.logical_shift_right)
            nc.vector.tensor_scalar(y.bitcast(I32), y.bitcast(I32), -1, 0x5F3759DF,
                                    op0=OP.mult, op1=OP.add)
            hh = pool.tile(shp, F32, tag="rsq_h", name=f"irsqh{n}")
            nc.vector.tensor_scalar_mul(hh, s, -0.5)
            for it in range(3):
                t = pool.tile(shp, F32, tag="rsq_t", name=f"irsqt{n}_{it}")
                nc.vector.tensor_tensor(out=t, in0=y, in1=y, op=OP.mult)
                nc.vector.tensor_tensor(out=t, in0=t, in1=hh, op=OP.mult)
                nc.vector.tensor_scalar_add(t, t, 1.5)
                yn = pool.tile(shp, F32, tag="rsq_y", name=f"irsqy{n}_{it}")
                nc.vector.tensor_tensor(out=yn, in0=y, in1=t, op=OP.mult)
                y = yn
            nc.vector.tensor_copy(out, y)

        def altcopy(dst, src):
            # alternate psum->sbuf evacuation between ACT and DVE
            if cpy_ctr[0] % 2 == 0:
                nc.scalar.copy(dst, src)
            else:
                nc.vector.tensor_copy(dst, src)
            cpy_ctr[0] += 1

        # ---------------- constants ----------------
        eye_f = consts.tile([128, 128], F32)
        nc.sync.dma_start(out=eye_f, in_=eye_d[:, :])
        eye_r = consts.tile([128, 128], F32R)
        nc.vector.tensor_copy(eye_r, eye_f)
        eye_b = consts.tile([128, 128], BF16)
        nc.vector.tensor_copy(eye_b, eye_f)
        lng_b = consts.tile([128, HD], F32)
        nc.gpsimd.dma_start(out=lng_b, in_=bass.AP(lng_d, 0, [[0, 128], [1, HD]]))
        lnb_b = consts.tile([128, HD], F32)
        nc.gpsimd.dma_start(out=lnb_b, in_=bass.AP(lnb_d, 0, [[0, 128], [1, HD]]))
        ones_col = consts.tile([128, 1], F32)
        nc.vector.memset(ones_col, 1.0)
        ones_row = consts.tile([1, 128], F32)
        nc.vector.memset(ones_row, 1.0)
        ones_row_r = consts.tile([1, 128], F32R)
        nc.vector.tensor_copy(ones_row_r, ones_row)
        ones_row4 = consts.tile([1, 4, 128], F32)
        nc.vector.memset(ones_row4, 1.0)
        ones_row4_r = consts.tile([1, 4, 128], BF16)
        nc.vector.tensor_copy(ones_row4_r, ones_row4)
        bs64_f = consts.tile([1, 64], F32)
        nc.sync.dma_start(out=bs64_f, in_=bs64_d[:, :])
        bs64_r = consts.tile([1, 64], BF16)
        nc.vector.tensor_copy(bs64_r, bs64_f)
        blr_bias = consts.tile([128, 1], F32)
        nc.vector.memset(blr_bias, BASE_LR_INV)
        eps6 = consts.tile([128, 1], F32)
        nc.vector.memset(eps6, 1e-6)
        lng_bb = consts.tile([128, HD], BF16)
        nc.vector.tensor_copy(lng_bb, lng_b)
        lnb_bb = consts.tile([128, HD], BF16)
        nc.vector.tensor_copy(lnb_bb, lnb_b)

        # wo (256, 2048) -> (128, 2, 2048) bf16
        wo_sb = consts.tile([128, 2, DIM], BF16)
        nc.gpsimd.dma_start(out=wo_sb, in_=wo_d.ap().rearrange("(j p) n -> p j n", p=128))

        # ---------------- fast-weight state ----------------
        # w_sb[j]: (128, 3, 128) [w0, w1, w2] rows-major f32r
        # wT_sb[j]: (128, 3, 128) [w0T, w2T, w1T] f32r
        # n0_sb[j]: (128, 3) initial row norms f32
        # w_bf[j] bf16 moving-operand copies: [w0T, w2T, w1T, w1(rows)]
        w_sb, wT_sb, n0_sb, w_bf = [], [], [], []
        for j in range(2):
            w_sb.append(consts.tile([128, 3, HD], F32R, tag=f"wsb{j}", name=f"wsb{j}"))
            wT_sb.append(consts.tile([128, 3, HD], F32R, tag=f"wTsb{j}", name=f"wTsb{j}"))
            n0_sb.append(consts.tile([128, 3], F32, tag=f"n0sb{j}", name=f"n0sb{j}"))
            w_bf.append(consts.tile([128, 4, HD], BF16, tag=f"wbf{j}", name=f"wbf{j}"))
        with tc.tile_pool(name="winit", bufs=2) as winit, \
             tc.tile_pool(name="winit_ps", bufs=2, space="PSUM") as winit_ps:
            for wi, wd in enumerate((w0_d, w1_d, w2_d)):
                wt = winit.tile([128, 2, HD], F32, tag="wload")
                nc.sync.dma_start(out=wt, in_=wd.ap().rearrange("(j p) n -> p j n", p=128))
                for j in range(2):
                    # initial row norms
                    scr = scr128.tile([128, HD], F32, tag="wscr")
                    ss = smalls.tile([128, 1], F32, tag="wss")
                    nc.scalar.activation(scr, wt[:, j, :], AF.Square, accum_out=ss)
                    r0 = smalls.tile([128, 1], F32, tag="wr0", name=f"wr0_{wi}_{j}")
                    emit_rsqrt(smalls, r0, ss)
                    nc.vector.tensor_tensor(out=n0_sb[j][:, wi : wi + 1], in0=ss,
                                            in1=r0, op=OP.mult)
                    # rounded state
                    nc.vector.tensor_copy(w_sb[j][:, wi, :], wt[:, j, :])
                    if wi == 1:
                        nc.vector.tensor_copy(w_bf[j][:, 3, :], wt[:, j, :])
                    # transpose
                    ps = winit_ps.tile([128, HD], F32R, tag="wtp")
                    nc.tensor.transpose(ps, w_sb[j][:, wi, :], eye_r)
                    slot = {0: 0, 2: 1, 1: 2}[wi]
                    altcopy(wT_sb[j][:, slot, :], ps)
                    nc.vector.tensor_copy(w_bf[j][:, slot, :], wT_sb[j][:, slot, :])

        # ================= PHASE A =================
        _sk = os.environ.get
        with ExitStack() as phA:
            xpool = phA.enter_context(tc.tile_pool(name="xpool", bufs=3))
            xT_pool = phA.enter_context(tc.tile_pool(name="xTpool", bufs=1))
            xT = xT_pool.tile([128, NT, 16, 128], BF16)
            wsm_sb = xT_pool.tile([128, 16, 64], BF16, name="wsm_sb")
            nc.gpsimd.dma_start(out=wsm_sb, in_=wsm_d.ap().rearrange("(k p) n -> p k n", p=128))

            with tc.tile_pool(name="tpA_ps", bufs=2, space="PSUM") as tpA_ps, \
                 tc.tile_pool(name="thin_ps", bufs=2, space="PSUM") as thin_ps, \
                 tc.tile_pool(name="thin_sb", bufs=2) as thin_sb:
                for t in range(NT) if _sk("KSKIP_PHA") != "1" else []:
                    xt = xpool.tile([128, DIM], BF16, tag="xt")
                    nc.sync.dma_start(out=xt, in_=xs_d[t * 128 : (t + 1) * 128, :])
                    for kb in range(4):
                        ps = tpA_ps.tile([128, 4, 128], BF16, tag="tp")
                        for r in range(4):
                            k = 4 * kb + r
                            nc.tensor.transpose(ps[:, r, :],
                                                xt[:, k * 128 : (k + 1) * 128], eye_b)
                        altcopy(xT[:, t, 4 * kb : 4 * kb + 4, :], ps)
                # thin projection (feature-major): psum (64, 512) per 4-tile group
                for tg in range(2) if _sk("KSKIP_PHA") != "1" else []:
                    pst = thin_ps.tile([64, 4, 128], F32, tag="thin")
                    for k in range(16):
                        nc.tensor.matmul(pst, wsm_sb[:, k, :],
                                         xT[:, 4 * tg : 4 * (tg + 1), k, :],
                                         start=(k == 0), stop=False)
                    # + b_s (row-constant bias on scale rows; zero elsewhere)
                    nc.tensor.matmul(pst, bs64_r, ones_row4_r, start=False, stop=True)
                    tsb = thin_sb.tile([64, 4, 128], F32, tag="tsb")
                    altcopy(tsb, pst)
                    nc.sync.dma_start(out=cc_in[:, 512 * tg : 512 * (tg + 1)], in_=tsb)
                if os.environ.get("KNOCC") == "1":
                    # timing-model build: collective replaced by a local DRAM copy
                    nc.sync.dma_start(out=cc_out[:, :], in_=cc_in[:, :])
                else:
                    nc.gpsimd.collective_compute(
                        "AllToAll", OP.bypass,
                        replica_groups=[list(range(NCORES))],
                        ins=[cc_in.ap()], outs=[cc_out.ap()],
                    )
                if dbg:
                    nc.sync.dma_start(out=dbg_ccin[:, :], in_=cc_in[:, :])
                    nc.sync.dma_start(out=dbg_ccout[:, :], in_=cc_out[:, :])

            # qkv projection: loop over head column blocks
            with tc.tile_pool(name="wqpool", bufs=2) as wqpool, \
                 tc.tile_pool(name="qkv_ps", bufs=4, space="PSUM") as qkv_ps, \
                 tc.tile_pool(name="stpool", bufs=3) as stpool:
                wq_r = wq_d.ap().rearrange("(k p) n -> p k n", p=128)
                for m in range(H) if _sk("KSKIP_PHA") != "1" else []:
                    wq_m = wqpool.tile([128, 16, 3 * HD], BF16, tag="wqm")
                    nc.sync.dma_start(out=wq_m, in_=wq_r[:, :, m * 384 : (m + 1) * 384])
                    for t in range(NT):
                        psq = qkv_ps.tile([128, 3 * HD], F32, tag="psq")
                        for k in range(16):
                            nc.tensor.matmul(psq, xT[:, t, k, :], wq_m[:, k, :],
                                             start=(k == 0), stop=(k == 15))
                        st = stpool.tile([128, 3 * HD], BF16, tag="st")
                        nc.scalar.activation(st, psq, AF.Silu)
                        nc.sync.dma_start(
                            out=stage_d[t, :, m * 384 : (m + 1) * 384], in_=st)

        # ================= LaCT rounds =================
        # PSUM budget (8 banks, bank-quantized): fw 2 + bwd 2 + ns 2 + tp 1 + sm 1
        with tc.tile_pool(name="lrpool", bufs=2) as lrpool, \
             tc.tile_pool(name="qkvt_pool", bufs=2) as qkvt_pool, \
             tc.tile_pool(name="prep_pool", bufs=2) as prep_pool, \
             tc.tile_pool(name="fwd_pool", bufs=2) as fwd_pool, \
             tc.tile_pool(name="bwd_pool", bufs=2) as bwd_pool, \
             tc.tile_pool(name="ns_pool", bufs=2) as ns_pool, \
             tc.tile_pool(name="owpool", bufs=2) as owpool, \
             tc.tile_pool(name="small2", bufs=4) as small2, \
             tc.tile_pool(name="ps_fw", bufs=2, space="PSUM") as ps_fw, \
             tc.tile_pool(name="ps_bwd", bufs=2, space="PSUM") as ps_bwd, \
             tc.tile_pool(name="ps_ns", bufs=2, space="PSUM") as ps_ns, \
             tc.tile_pool(name="ps_tp", bufs=1, space="PSUM") as ps_tp, \
             tc.tile_pool(name="ps_sm", bufs=1, space="PSUM") as ps_sm:

            uid = [0]

            def nm(s):
                uid[0] += 1
                return f"{s}{uid[0]}"

            rot = [0]

            def copy3(dst, src):
                # rotate psum->sbuf evacuations over ACT / DVE (Pool cannot
                # access PSUM on this hardware)
                r = rot[0] % 3
                rot[0] += 1
                if r == 2:
                    nc.vector.tensor_copy(dst, src)
                else:
                    nc.scalar.copy(dst, src)

            rot2 = [0]

            def stt2(out, in0, scalar, in1, op0, op1, psum=False):
                # rotate scalar_tensor_tensor over DVE / Pool; PSUM operands
                # must stay on DVE (Pool cannot access PSUM)
                if psum:
                    nc.vector.scalar_tensor_tensor(out=out, in0=in0, scalar=scalar,
                                                   in1=in1, op0=op0, op1=op1)
                    return
                nc.vector.scalar_tensor_tensor(out=out, in0=in0, scalar=scalar,
                                               in1=in1, op0=op0, op1=op1)

            rot4 = [0]

            def ts2(out, in0, s1):
                # rotate per-partition scalar multiplies DVE(x3) / Pool(x1)
                # TensorScalarPtr is DVE-only (Pool lacks the opcode)
                nc.vector.tensor_scalar_mul(out, in0, s1)

            rsq_ctr = [0]

            def rsqrt_dve(out, sv):
                # 1/sqrt(sv) on DVE: quake seed + 3 Newton iterations
                I32 = mybir.dt.int32
                n = rsq_ctr[0]
                rsq_ctr[0] += 1
                shp = list(sv.shape)
                y = small2.tile(shp, F32, tag="rsq_y", name=f"rsqy{n}")
                nc.vector.tensor_scalar(y.bitcast(I32), sv.bitcast(I32), 1, None,
                                        op0=OP.logical_shift_right)
                nc.vector.tensor_scalar(y.bitcast(I32), y.bitcast(I32), -1,
                                        0x5F3759DF, op0=OP.mult, op1=OP.add)
                hh = small2.tile(shp, F32, tag="rsq_h", name=f"rsqh{n}")
                nc.vector.tensor_scalar_mul(hh, sv, -0.5)
                for it in range(3):
                    t = small2.tile(shp, F32, tag="rsq_t", name=f"rsqt{n}_{it}")
                    nc.vector.tensor_tensor(out=t, in0=y, in1=y, op=OP.mult)
                    nc.vector.tensor_tensor(out=t, in0=t, in1=hh, op=OP.mult)
                    nc.vector.tensor_scalar_add(t, t, 1.5)
                    yn = small2.tile(shp, F32, tag="rsq_y", name=f"rsqy{n}_{it}")
                    nc.vector.tensor_tensor(out=yn, in0=y, in1=t, op=OP.mult)
                    y = yn
                nc.vector.tensor_copy(out, y)

            mark_tiles = []

            def mark(label):
                if os.environ.get("KMARK") == "1":
                    mt = small2.tile([1, 1], F32, tag="mark", name=nm("mark"), bufs=1)
                    nc.vector.memset(mt, float(len(mark_tiles)))
                    mark_tiles.append(label)

            # ---- learning rates / scales for all 4 rounds ----
            lrvs, lrts = [], []
            for ci in range(4):
                lrt = lrpool.tile([128, 8, 16], F32, tag="lrt", name=nm("lrt"),
                                  bufs=4)
                lrts.append(lrt)
                for b in range(2):
                    src = bass.AP(cc_out, 16384 * ci + 8192 * b,
                                  [[16, 64], [1024, 8], [1, 16]])
                    nc.sync.dma_start(out=lrt[64 * b : 64 * (b + 1), :, :], in_=src)
                lrv = lrpool.tile([128, 8, 16], F32, tag=f"lrv{ci}", name=f"lrv{ci}",
                                  bufs=1)
                # softplus(z) = ln(1 + e^z); z = raw + base_lr_inv (z in ~[-8, 8])
                ez = lrpool.tile([128, 6, 16], F32, tag="ez", name=nm("ez"))
                nc.scalar.activation(ez, lrt[:, 0:6, :], AF.Exp, bias=blr_bias)
                nc.scalar.activation(lrv[:, 0:6, :], ez, AF.Ln, bias=ones_col)
                lrvs.append(lrv)
            for ci in range(4):
                nc.scalar.activation(lrvs[ci][:, 6:8, :],
                                     lrts[ci][:, 6:8, :], AF.Silu)

            # ---- per-round tile handles ----
            cur = {}   # (ci, j) -> dict of tiles

            def prep_pieces(ci, j):
                """Emit-thunks preparing round ci group j inputs: load, l2-norm,
                scale+cast, transposes. Returns list of zero-arg thunks."""
                st = {}
                cur[(ci, j)] = st
                tidx = j * 4 + ci

                def p_load():
                    qkvt = qkvt_pool.tile([128, H, 3 * HD], BF16, tag="qkvt",
                                          name=nm("qkvt"))
                    st["qkvt"] = qkvt
                    nc.sync.dma_start(out=qkvt, in_=stage_d[tidx])

                def p_norm():
                    qkvt = st["qkvt"]
                    sq = prep_pool.tile([128, H, 128], BF16, tag="sq", name=nm("sq"),
                                        bufs=1)
                    nrm = small2.tile([128, 2, 16], F32, tag="nrm", name=nm("nrm"))
                    nc.scalar.activation(sq, qkvt[:, :, 0:128], AF.Square)
                    nc.vector.tensor_reduce(nrm[:, 0, :], sq, mybir.AxisListType.X,
                                            OP.add)
                    sqk = prep_pool.tile([128, H, 128], BF16, tag="sq", name=nm("sqk"),
                                         bufs=1)
                    nc.scalar.activation(sqk, qkvt[:, :, 128:256], AF.Square)
                    nc.vector.tensor_reduce(nrm[:, 1, :], sqk, mybir.AxisListType.X,
                                            OP.add)
                    nfac = small2.tile([128, 2, 16], F32, tag="nfac", name=nm("nfac"))
                    rsqrt_dve(nfac, nrm)
                    st["nfac"] = nfac

                def p_scale():
                    qkvt, nfac = st["qkvt"], st["nfac"]
                    for m in range(H):
                        ts2(qkvt[:, m, 0:128], qkvt[:, m, 0:128],
                            nfac[:, 0, m : m + 1])
                        ts2(qkvt[:, m, 128:256], qkvt[:, m, 128:256],
                            nfac[:, 1, m : m + 1])

                def mk_tp(srcf, dstkey):
                    def p_tp():
                        dst = prep_pool.tile([128, H, 128], BF16, tag=dstkey,
                                             name=nm(dstkey))
                        st[dstkey] = dst
                        for b in range(4):
                            tp = ps_tp.tile([128, 4, 128], BF16, tag="tp",
                                            name=nm("tp"))
                            for r in range(4):
                                m = 4 * b + r
                                nc.tensor.transpose(tp[:, r, :], srcf(m), eye_b)
                            copy3(dst[:, 4 * b : 4 * b + 4, :], tp)
                    return p_tp

                return [p_load, p_norm, p_scale,
                        mk_tp(lambda m: st["qkvt"][:, m, 0:128], "qT"),
                        mk_tp(lambda m: st["qkvt"][:, m, 128:256], "kT"),
                        mk_tp(lambda m: st["qkvt"][:, m, 256:384], "vT")]

            def emit_fwd(ci, blk_hook=None):
                lrv = lrvs[ci]
                for j in range(2):
                    st = cur[(ci, j)]
                    st["gh"] = fwd_pool.tile([128, H, HD], BF16, tag="gh",
                                             name=nm("gh"))
                for s in range(4):
                    for j in range(2):
                        st = cur[(ci, j)]
                        qT, gh = st["qT"], st["gh"]
                        psg = ps_fw.tile([128, 4, 128], F32, tag="fw", name=nm("psg"))
                        psh = ps_fw.tile([128, 4, 128], F32, tag="fw", name=nm("psh"))
                        rhs = qT[:, 4 * s : 4 * (s + 1), :]
                        nc.tensor.matmul(psg, w_bf[j][:, 0, :], rhs, start=True,
                                         stop=True)
                        nc.tensor.matmul(psh, w_bf[j][:, 1, :], rhs, start=True,
                                         stop=True)
                        sg4 = fwd_pool.tile([128, 4, 128], BF16, tag="sg4",
                                            name=nm("sg4"))
                        nc.scalar.activation(sg4, psg, AF.Silu)
                        nc.vector.tensor_tensor(out=gh[:, 4 * s : 4 * (s + 1), :],
                                                in0=sg4, in1=psh, op=OP.mult)
                # second layer + LN stats (quad psum per 4 heads)
                for j in range(2):
                    st = cur[(ci, j)]
                    st["mvb"] = small2.tile([128, H, 2], F32, tag="mvb",
                                            name=nm("mvb"))
                    st["outn"] = fwd_pool.tile([128, H, HD], BF16, tag="outn",
                                               name=nm("outn"))
                for b in range(4):
                    for j in range(2):
                        st = cur[(ci, j)]
                        pso = ps_fw.tile([128, 4, 128], F32, tag="fw", name=nm("pso"))
                        for r in range(4):
                            m = 4 * b + r
                            nc.tensor.matmul(pso[:, r, :], st["gh"][:, m, :],
                                             w_bf[j][:, 2, :], start=True, stop=True)
                        for r in range(4):
                            m = 4 * b + r
                            st6 = small2.tile([128, 6], F32, tag="st6",
                                              name=nm("st6"))
                            nc.vector.bn_stats(st6, pso[:, r, :])
                            nc.vector.bn_aggr(st["mvb"][:, m, :], st6)
                            nc.vector.tensor_scalar(st["outn"][:, m, :], pso[:, r, :],
                                                    st["mvb"][:, m, 0:1], None,
                                                    op0=OP.subtract)
                for j in range(2):
                    st = cur[(ci, j)]
                    rstdb = small2.tile([128, H], F32, tag="rstdb", name=nm("rstdb"))
                    vpe = small2.tile([128, H], F32, tag="vpe", name=nm("vpe"))
                    nc.vector.tensor_scalar_add(vpe, st["mvb"][:, :, 1], 1e-6)
                    rsqrt_dve(rstdb, vpe)
                    st["rstdb"] = rstdb
                    st["outT"] = fwd_pool.tile([128, H, HD], BF16, tag="outT",
                                               name=nm("outT"))
                # LN apply + scale + transpose
                for b in range(4):
                    for j in range(2):
                        st = cur[(ci, j)]
                        tp = ps_tp.tile([128, 4, 128], BF16, tag="tp", name=nm("tpo"))
                        for r in range(4):
                            m = 4 * b + r
                            t2 = fwd_pool.tile([128, 128], BF16, tag="t2",
                                               name=nm("t2"))
                            nc.vector.tensor_scalar(t2, st["outn"][:, m, :],
                                                    st["rstdb"][:, m : m + 1],
                                                    lrv[:, 6 + j, m : m + 1],
                                                    op0=OP.mult, op1=OP.mult)
                            t3 = fwd_pool.tile([128, 128], BF16, tag="t3",
                                               name=nm("t3"))
                            nc.vector.tensor_tensor(out=t3, in0=t2, in1=lng_bb,
                                                    op=OP.mult)
                            tns = fwd_pool.tile([128, 128], BF16, tag="tns",
                                                name=nm("tns"))
                            stt2(tns, lnb_bb, lrv[:, 6 + j, m : m + 1], t3,
                                 OP.mult, OP.add)
                            nc.tensor.transpose(tp[:, r, :], tns, eye_b)
                        copy3(st["outT"][:, 4 * b : 4 * b + 4, :], tp)
                    if blk_hook is not None:
                        blk_hook(b)

            def emit_bwd(ci):
                lrv = lrvs[ci]
                for j in range(2):
                    st = cur[(ci, j)]
                    for key in ("dgb_lr", "hid_lr", "dhb_lr"):
                        st[key] = bwd_pool.tile([128, H, HD], BF16, tag=key,
                                                name=nm(key))
                for q in range(4):
                    for j in range(2):
                        st = cur[(ci, j)]
                        kT, vT = st["kT"], st["vT"]
                        ms = range(4 * q, 4 * q + 4)
                        GB = ps_bwd.tile([128, 4, 128], F32, tag="bwd", name=nm("GB"))
                        for r, m in enumerate(ms):
                            nc.tensor.matmul(GB[:, r, :], kT[:, m, :],
                                             w_bf[j][:, 0, :], start=True, stop=True)
                        # sig = sigmoid(GB) via tanh; sg = silu(GB);
                        # dsilu = sig + sg - sg*sig  (shallow, no GB re-read)
                        th = bwd_pool.tile([128, 4, 128], BF16, tag="th", name=nm("th"))
                        nc.scalar.activation(th, GB, AF.Tanh, scale=0.5)
                        sg = bwd_pool.tile([128, 4, 128], BF16, tag="sg", name=nm("sg"))
                        nc.scalar.activation(sg, GB, AF.Silu)
                        sig = th
                        nc.vector.tensor_scalar(sig, th, 0.5, 0.5, op0=OP.mult,
                                                op1=OP.add)
                        bb = bwd_pool.tile([128, 4, 128], BF16, tag="bb", name=nm("bb"))
                        nc.gpsimd.tensor_tensor(out=bb, in0=sg, in1=sig, op=OP.mult)
                        cc = bwd_pool.tile([128, 4, 128], BF16, tag="cc", name=nm("cc"))
                        nc.vector.tensor_tensor(out=cc, in0=sig, in1=sg, op=OP.add)
                        dsl = bwd_pool.tile([128, 4, 128], BF16, tag="dsl",
                                            name=nm("dsl"))
                        nc.vector.tensor_tensor(out=dsl, in0=cc, in1=bb,
                                                op=OP.subtract)
                        HB = ps_bwd.tile([128, 4, 128], F32, tag="bwd", name=nm("HB"))
                        for r, m in enumerate(ms):
                            nc.tensor.matmul(HB[:, r, :], kT[:, m, :],
                                             w_bf[j][:, 1, :], start=True, stop=True)
                        HBs = bwd_pool.tile([128, 4, 128], BF16, tag="HBs",
                                            name=nm("HBs"))
                        nc.scalar.copy(HBs, HB)
                        hid = bwd_pool.tile([128, 4, 128], BF16, tag="hid",
                                            name=nm("hid"))
                        nc.gpsimd.tensor_tensor(out=hid, in0=sg, in1=HBs, op=OP.mult)
                        dgbA = bwd_pool.tile([128, 4, 128], BF16, tag="dgbA",
                                             name=nm("dgbA"))
                        nc.gpsimd.tensor_tensor(out=dgbA, in0=HBs, in1=dsl,
                                                op=OP.mult)
                        DH = ps_bwd.tile([128, 4, 128], F32, tag="bwd", name=nm("DH"))
                        for r, m in enumerate(ms):
                            nc.tensor.matmul(DH[:, r, :], vT[:, m, :],
                                             w_bf[j][:, 3, :], start=True, stop=True)
                        DHs = bwd_pool.tile([128, 4, 128], BF16, tag="DHs",
                                            name=nm("DHs"))
                        nc.scalar.copy(DHs, DH)
                        dhbA = bwd_pool.tile([128, 4, 128], BF16, tag="dhbA",
                                             name=nm("dhbA"))
                        nc.gpsimd.tensor_tensor(out=dhbA, in0=DHs, in1=sg, op=OP.mult)
                        dgbB = dgbA
                        nc.gpsimd.tensor_tensor(out=dgbB, in0=dgbA, in1=DHs,
                                                op=OP.mult)
                        for r, m in enumerate(ms):
                            ts2(st["dgb_lr"][:, m, :], dgbB[:, r, :],
                                lrv[:, 3 * j + 0, m : m + 1])
                            ts2(st["hid_lr"][:, m, :], hid[:, r, :],
                                lrv[:, 3 * j + 1, m : m + 1])
                            ts2(st["dhb_lr"][:, m, :], dhbA[:, r, :],
                                lrv[:, 3 * j + 2, m : m + 1])
                # dw accumulation (3 slices of one bank) + X/XT triples
                for j in range(2):
                    st = cur[(ci, j)]
                    qkvt = st["qkvt"]
                    dwt = ps_sm.tile([128, 4, 128], F32, tag="sm", name=nm("dwt"))
                    for wi in range(3):
                        for m in range(H):
                            if wi == 0:
                                nc.tensor.matmul(dwt[:, 0, :], st["dgb_lr"][:, m, :],
                                                 qkvt[:, m, 128:256], start=(m == 0),
                                                 stop=(m == 15))
                            elif wi == 1:
                                nc.tensor.matmul(dwt[:, 1, :], qkvt[:, m, 256:384],
                                                 st["hid_lr"][:, m, :],
                                                 start=(m == 0), stop=(m == 15))
                            else:
                                nc.tensor.matmul(dwt[:, 2, :], st["dhb_lr"][:, m, :],
                                                 qkvt[:, m, 128:256], start=(m == 0),
                                                 stop=(m == 15))
                    ssf = small2.tile([128, 3], F32, tag="ssf", name=nm("ssf"))
                    for wi in range(3):
                        scr = small2.tile([128, HD], F32, tag="nsscr", name=nm("scr"),
                                          bufs=2)
                        nc.scalar.activation(scr, dwt[:, wi, :], AF.Square,
                                             accum_out=ssf[:, wi : wi + 1])
                    nc.tensor.matmul(dwt[0:1, 3, 0:3], ones_col, ssf, start=True,
                                     stop=True)
                    r13 = small2.tile([1, 3], F32, tag="r13", name=nm("r13"))
                    s13 = small2.tile([1, 3], F32, tag="s13", name=nm("s13"))
                    nc.vector.tensor_copy(s13, dwt[0:1, 3, 0:3])
                    rsqrt_dve(r13, s13)
                    nc.tensor.matmul(dwt[:, 3, 4:7], ones_row, r13, start=True,
                                     stop=True)
                    rbs = small2.tile([128, 3], F32, tag="rbs", name=nm("rbs"))
                    nc.scalar.copy(rbs, dwt[:, 3, 4:7])
                    Xt = ns_pool.tile([128, 3, HD], BF16, tag=f"X{j}", name=nm("Xt"))
                    for wi in range(3):
                        nc.vector.tensor_scalar_mul(Xt[:, wi, :], dwt[:, wi, :],
                                                    rbs[:, wi : wi + 1])
                    tpx = ps_tp.tile([128, 4, 128], BF16, tag="tp", name=nm("tpx"))
                    for wi in range(3):
                        nc.tensor.transpose(tpx[:, wi, :], Xt[:, wi, :], eye_b)
                    XTt = ns_pool.tile([128, 3, HD], BF16, tag=f"XT{j}",
                                       name=nm("XTt"))
                    copy3(XTt, tpx[:, 0:3, :])
                    st["X"], st["XT"] = Xt, XTt

            def ns_step(ci, j, it):
                # one Newton-Schulz iteration for all 3 weights of group j
                st = cur[(ci, j)]
                X, XT = st["X"], st["XT"]
                a_i, b_i, c_i = NS_COEFS[it]
                last = it == len(NS_COEFS) - 1
                psA = ps_ns.tile([128, 3, HD], F32, tag="ns", name=nm("psA"))
                for wi in range(3):
                    nc.tensor.matmul(psA[:, wi, :], XT[:, wi, :], XT[:, wi, :],
                                     start=True, stop=True)
                A_sb = ns_pool.tile([128, 3, HD], BF16, tag=f"A{j}", name=nm("Asb"),
                                    bufs=1)
                copy3(A_sb, psA)
                psA2 = ps_ns.tile([128, 3, HD], F32, tag="ns", name=nm("psA2"))
                for wi in range(3):
                    nc.tensor.matmul(psA2[:, wi, :], A_sb[:, wi, :], A_sb[:, wi, :],
                                     start=True, stop=True)
                t1 = ns_pool.tile([128, 3, HD], BF16, tag=f"t1{j}", name=nm("t1"),
                                  bufs=1)
                nc.vector.tensor_scalar_mul(t1, psA2, c_i)
                Bm = ns_pool.tile([128, 3, HD], BF16, tag=f"Bm{j}", name=nm("Bm"),
                                  bufs=1)
                stt2(Bm, psA, b_i, t1, OP.mult, OP.add, psum=True)
                psBX = ps_ns.tile([128, 3, HD], F32, tag="ns", name=nm("psBX"))
                for wi in range(3):
                    nc.tensor.matmul(psBX[:, wi, :], Bm[:, wi, :], X[:, wi, :],
                                     start=True, stop=True)
                Xn = ns_pool.tile([128, 3, HD], BF16, tag=f"X{j}", name=nm("Xn"))
                stt2(Xn, X, a_i, psBX, OP.mult, OP.add, psum=True)
                st["X"] = Xn
                if not last:
                    psXTB = ps_ns.tile([128, 3, HD], F32, tag="ns", name=nm("psXTB"))
                    for wi in range(3):
                        nc.tensor.matmul(psXTB[:, wi, :], X[:, wi, :], Bm[:, wi, :],
                                         start=True, stop=True)
                    XTn = ns_pool.tile([128, 3, HD], BF16, tag=f"XT{j}",
                                       name=nm("XTn"))
                    stt2(XTn, XT, a_i, psXTB, OP.mult, OP.add, psum=True)
                    st["XT"] = XTn
                else:
                    # w += X; renormalize rows; refresh f32r + bf16 copies
                    wtmps = []
                    ss3 = small2.tile([128, 3], F32, tag="ss3", name=nm("ss3"))
                    for wi in range(3):
                        wtmp = small2.tile([128, HD], F32, tag=f"wtmp{wi}",
                                           name=nm("wtmp"), bufs=2)
                        nc.vector.tensor_tensor(out=wtmp, in0=w_sb[j][:, wi, :],
                                                in1=Xn[:, wi, :], op=OP.add)
                        wtmps.append(wtmp)
                        scr2 = small2.tile([128, HD], F32, tag="nsscr",
                                           name=nm("scr2"), bufs=2)
                        nc.scalar.activation(scr2, wtmp, AF.Square,
                                             accum_out=ss3[:, wi : wi + 1])
                    rc3 = small2.tile([128, 3], F32, tag="rc3", name=nm("rc3"))
                    rsqrt_dve(rc3, ss3)
                    fac3 = small2.tile([128, 3], F32, tag="fac3", name=nm("fac3"))
                    nc.vector.tensor_tensor(out=fac3, in0=rc3, in1=n0_sb[j],
                                            op=OP.mult)
                    for wi in range(3):
                        nc.vector.tensor_scalar_mul(w_sb[j][:, wi, :], wtmps[wi],
                                                    fac3[:, wi : wi + 1])
                        if wi == 1:
                            nc.vector.tensor_scalar_mul(w_bf[j][:, 3, :], wtmps[wi],
                                                        fac3[:, wi : wi + 1])
                    pswt = ps_sm.tile([128, 4, 128], F32, tag="sm", name=nm("pswt"))
                    for wi in range(3):
                        nc.tensor.transpose(pswt[:, wi, :].bitcast(F32R),
                                            w_sb[j][:, wi, :], eye_r)
                    for wi in range(3):
                        slot = {0: 0, 2: 1, 1: 2}[wi]
                        copy3(wT_sb[j][:, slot, :], pswt[:, wi, :].bitcast(F32R))
                        nc.gpsimd.tensor_copy(w_bf[j][:, slot, :],
                                              wT_sb[j][:, slot, :])

            def wo_piece(ci, m):
                ow = owpool.tile([128, DIM], BF16, tag="ow", name=nm("ow"))
                for n in range(4):
                    psw = ps_bwd.tile([128, 4, 128], F32, tag="bwd", name=nm("psw"))
                    for j in range(2):
                        nc.tensor.matmul(psw, cur[(ci, j)]["outT"][:, m, :],
                                         wo_sb[:, j, n * 512 : (n + 1) * 512],
                                         start=(j == 0), stop=(j == 1))
                    copy3(ow[:, n * 512 : (n + 1) * 512], psw)
                dst = bass.AP(out_d, (2048 * ci + m) * DIM, [[16 * DIM, 128], [1, DIM]])
                nc.sync.dma_start(out=dst, in_=ow)

            # ---------------- main schedule ----------------
            for p in prep_pieces(0, 0):
                p()
            for p in prep_pieces(0, 1):
                p()
            for ci in range(4):
                mark(f"fwd{ci}")
                if ci == 3:
                    emit_fwd(ci, blk_hook=lambda b: [wo_piece(3, m)
                                                     for m in range(4 * b, 4 * b + 4)])
                else:
                    emit_fwd(ci)
                if ci < 3:
                    mark(f"bwd{ci}")
                    emit_bwd(ci)
                    fillers = [lambda m=m: wo_piece(ci, m) for m in range(H)]
                    nxt0 = prep_pieces(ci + 1, 0)
                    nxt1 = prep_pieces(ci + 1, 1)
                    fillers += [t for pair in zip(nxt0, nxt1) for t in pair]
                    fi = 0
                    for it in range(len(NS_COEFS)):
                        for j in range(2):
                            ns_step(ci, j, it)
                            for _ in range(3):
                                if fi < len(fillers):
                                    fillers[fi]()
                                    fi += 1
                    mark(f"nsdone{ci}")
                    while fi < len(fillers):
                        fillers[fi]()
                        fi += 1
                    mark(f"fill{ci}")
                else:
                    pass  # W_o(3) interleaved into emit_fwd via blk_hook

    if os.environ.get("KSKIPCOMPILE") != "1":
        nc.compile()
    return nc


def _shard_inputs(x, W_qkv, W_lr, W_o, W_s, b_s, ln_g, ln_b, w0, w1, w2):
    import ml_dtypes
    x = np.ascontiguousarray(np.asarray(x, dtype=np.float32).reshape(L, DIM)
                             .astype(np.float16))
    W_qkv = np.ascontiguousarray(np.asarray(W_qkv, dtype=np.float32)
                                 .astype(np.float16))
    W_lr = np.asarray(W_lr, dtype=np.float32)
    W_s = np.asarray(W_s, dtype=np.float32)
    b_s = np.asarray(b_s, dtype=np.float32)
    W_o = np.asarray(W_o, dtype=np.float32)
    ln_g = np.ascontiguousarray(np.asarray(ln_g, dtype=np.float32).reshape(1, HD))
    ln_b = np.ascontiguousarray(np.asarray(ln_b, dtype=np.float32).reshape(1, HD))
    w0 = np.asarray(w0, dtype=np.float32)
    w1 = np.asarray(w1, dtype=np.float32)
    w2 = np.asarray(w2, dtype=np.float32)
    eye = np.eye(128, dtype=np.float32)

    wsm_full = np.concatenate([W_lr, W_s], axis=1)  # (2048, 64)
    cols = []
    bs64 = np.zeros((1, 64), dtype=np.float32)
    for c in range(NCORES):
        g0, g1 = 2 * c, 2 * c + 1
        blk = [g0, 16 + g0, 32 + g0, g1, 16 + g1, 32 + g1, 48 + g0, 48 + g1]
        cols.extend(blk)
        bs64[0, 8 * c + 6] = b_s[g0]
        bs64[0, 8 * c + 7] = b_s[g1]
    wsm = np.ascontiguousarray(wsm_full[:, cols]).astype(np.float16)

    in_maps = []
    for c in range(NCORES):
        g0, g1 = 2 * c, 2 * c + 1
        in_maps.append({
            "xs": np.ascontiguousarray(x[1024 * c : 1024 * (c + 1)]),
            "wq": W_qkv,
            "wsm": wsm,
            "bs64": bs64,
            "wo": np.ascontiguousarray(W_o[np.r_[g0 * HD:(g0 + 1) * HD,
                                               g1 * HD:(g1 + 1) * HD], :]
                                       ).astype(np.float16),
            "w0i": np.ascontiguousarray(np.concatenate([w0[g0], w0[g1]], axis=0)),
            "w1i": np.ascontiguousarray(np.concatenate([w1[g0], w1[g1]], axis=0)),
            "w2i": np.ascontiguousarray(np.concatenate([w2[g0], w2[g1]], axis=0)),
            "lng": ln_g,
            "lnb": ln_b,
            "eye": eye,
        })
    return in_maps


def kernel(x, W_qkv, W_lr, W_o, W_m, b_m, W_s, b_s, ln_g, ln_b, w0, w1, w2):
    # W_m / b_m intentionally unused: momentum rescales dw by (1 + m) > 0 before
    # Newton-Schulz, which normalizes away any positive scalar factor.
    if "nc" not in _CACHED:
        _CACHED["nc"] = build_kernel()
    nc = _CACHED["nc"]
    in_maps = _shard_inputs(x, W_qkv, W_lr, W_o, W_s, b_s, ln_g, ln_b, w0, w1, w2)
    res = run_bass_kernel_spmd(nc, in_maps, core_ids=list(range(NCORES)))
    out = np.zeros((L, DIM), dtype=np.float64)
    for c in range(NCORES):
        out += res.results[c]["out"]
    return out.astype(np.float32).reshape(1, L, DIM)

